# revision 4
# baseline (speedup 1.0000x reference)
"""AffinityPropagate Trainium2 kernel, v3.

24 iterations of an 8-neighbor gated stencil:
    d <- (1-mask) * sum_k(gsh_k * shift_k(d)) / wsum + mask * blur

Strategy (8 NeuronCores, pure data parallel: one batch image per core):
  * Image [352, 1216] flattened row-major into SBUF [128 part x 3344].
  * Zero-padded 2D shifts == flat 1D shifted reads: per-direction gate
    weights are exactly 0 wherever a neighbor is out of bounds, so the
    row-wrap values the flat shift drags in are annihilated.
  * Per-direction weights W_k = |g_k shifted| * (1-mask)/wsum (fp16) are
    precomputed once into one [128, 8, F] tile.
  * The 8 shifts decompose into two rank-2 lattices, each coverable by a
    single 4-dim access pattern (middle dims of size 2x2):
      grpA: s = -1216 + 1215*a + 1217*b -> {-1216, -1, +1, +1216}
      grpB: s = -1217 +    2*a + 2432*b -> {-1217, -1215, +1215, +1217}
    so DVE computes grpA (4 planes) in ONE fp16-2x tensor_tensor per
    chunk, plus the left part of grpB; Pool (gpsimd) multiplies the
    right part of grpB (its Multiply runs at 0.42 roofline, so it gets
    ~42% of one group). PE accumulates the 8 product planes onto a
    b-seeded PSUM bank via identity matmuls; ACT seeds and casts.
  * grpB touches only far-shifted data, so at the chunk-order reversal
    between iterations it never waits on the just-written chunk; deep
    tile buffering (bufs=3 products) absorbs the remaining latency.
  * Preamble: 1/wsum on ACT (Reciprocal activation) + per-bank weight
    scaling interleaved with iteration-1 products so compute starts
    under the 47.6us input-DMA shadow.
"""

import numpy as np

import bass_rust
from concourse import bass, mybir
from concourse.bass_utils import run_bass_kernel_spmd
from concourse.tile import TileContext

B, H, W = 8, 352, 1216
HW = H * W            # 428032
P = 128
F = HW // P           # 3344
HALO = 1218           # > max |shift| (1217), even
DW = HALO + F + HALO  # d tile width
PROP_TIME = 24
NCHUNK = 4            # product chunks per iteration
CD = F // NCHUNK      # 836
CP = 418              # one PSUM bank
WE = 486              # DVE's share of grpB columns per chunk (Pool: rest)

# wt3 plane order (storage): grpA = planes 0-3, grpB = planes 4-7
WSHIFTS = [-1216, -1, 1, 1216, -1217, -1215, 1215, 1217]
# guidance channel k for each reference offset (dy*W+dx), reference order
REF_SHIFTS = [1217, 1216, 1215, 1, -1, -1215, -1216, -1217]
CHAN_FOR_SHIFT = {s: k for k, s in enumerate(REF_SHIFTS)}

f32 = mybir.dt.float32
f16 = mybir.dt.float16
MULT = mybir.AluOpType.mult
ADD = mybir.AluOpType.add

_CACHE = {}


def _split_sync_waits(nc, max_waits=1):
    """The walrus in this container accepts at most one sync-wait command
    per instruction; hoist extras onto preceding same-engine no-ops."""
    for f in nc.m.functions:
        for bb in f.blocks:
            out = []
            for inst in bb.instructions:
                si = inst.sync_info
                if si is not None and si.on_wait and len(si.on_wait) > max_waits:
                    waits = list(si.on_wait)
                    carry, keep = waits[:-max_waits], waits[-max_waits:]
                    for j, w in enumerate(carry):
                        out.append(mybir.InstNoOp(
                            name=f"{inst.name}-ws{j}", engine=inst.engine,
                            sync_info=mybir.SyncInfo(on_wait=[w], on_update=[]),
                            bass_nofuse=True))
                    inst.sync_info = mybir.SyncInfo(
                        on_wait=keep, on_update=list(si.on_update))
                out.append(inst)
            bb.instructions[:] = out


def _ap4(tile, base, d1, n1, d2, n2, w):
    """4-dim AP over `tile` starting at free offset `base`:
    [P, n1 (stride d1), n2 (stride d2), w (stride 1)]."""
    s = tile[:]
    return bass_rust.AP(tensor=s.tensor, offset=s.offset + base,
                        ap=[[s.ap[0][0], P], [d1, n1], [d2, n2], [1, w]])


def _bcast_k_ap(tile2d, start, width, nk):
    """AP reading tile2d[:, start:start+width] broadcast over a middle
    k-dim of size nk -> [P, nk, width] view."""
    s = tile2d[:, start:start + width]
    pstride = s.ap[0][0]
    return bass_rust.AP(tensor=s.tensor, offset=s.offset,
                        ap=[[pstride, P], [0, nk], [1, width]])


def _emit_shifted_plane_load(nc, gst, g, k, s, zrow):
    """gst[p, j] <- g[k, p*F + j + s], with rows whose 2D source row is out
    of bounds forced to zero (wrap columns are handled via mask multiplies)."""
    engs = (nc.sync, nc.scalar)
    eng = engs[k % 2]
    if s >= 0:
        for i, (p0, p1) in enumerate(((0, 32), (32, 64), (64, 96), (96, 127))):
            engs[(k + i) % 2].dma_start(
                out=gst[p0:p1, :],
                in_=g[k, s + p0 * F:s + p1 * F].rearrange(
                    "(p f) -> p f", p=p1 - p0))
        if s > 0:
            eng.dma_start(
                out=gst[127:128, 0:F - s],
                in_=g[k, s + 127 * F:HW].rearrange("(p f) -> p f", p=1))
        else:
            eng.dma_start(
                out=gst[127:128, :],
                in_=g[k, 127 * F:HW].rearrange("(p f) -> p f", p=1))
    else:
        a = -s
        eng.dma_start(
            out=gst[0:1, a:F],
            in_=g[k, 0:F - a].rearrange("(p f) -> p f", p=1))
        for i, (p0, p1) in enumerate(((1, 32), (32, 64), (64, 96), (96, 128))):
            engs[(k + i) % 2].dma_start(
                out=gst[p0:p1, :],
                in_=g[k, p0 * F - a:p1 * F - a].rearrange(
                    "(p f) -> p f", p=p1 - p0))
    # top/bottom image rows (dy out of bounds) + DMA-uncovered slivers.
    if s in (-1217, -1216, -1215):          # dy = -1
        nc.vector.memset(gst[0:1, 0:max(1216, -s)], 0.0)
    elif s == -1:
        nc.vector.memset(gst[0:1, 0:1], 0.0)
    elif s in (1215, 1216, 1217):           # dy = +1
        start = min(F - 1216, F - s)
        eng.dma_start(out=gst[127:128, start:F], in_=zrow[0:1, 0:F - start])
    elif s == 1:
        eng.dma_start(out=gst[127:128, F - 1:F], in_=zrow[0:1, 0:1])


def _build(prop_time=PROP_TIME):
    nc = bass.Bass()
    g = nc.dram_tensor("g", [8, HW], f32, kind="ExternalInput")
    blur = nc.dram_tensor("blur", [HW], f32, kind="ExternalInput")
    sparse = nc.dram_tensor("sparse", [HW], f32, kind="ExternalInput")
    ident = nc.dram_tensor("ident", [P, P], f16, kind="ExternalInput")
    out = nc.dram_tensor("out", [P, F], f32, kind="ExternalOutput")

    with TileContext(nc) as tc:
        with tc.tile_pool(name="const", bufs=1) as constp, \
             tc.tile_pool(name="wpool", bufs=1) as wpool, \
             tc.tile_pool(name="dpool", bufs=1) as dpool, \
             tc.tile_pool(name="misc", bufs=1) as miscp:

            identt = constp.tile([P, P], f16)
            nc.sync.dma_start(out=identt[:], in_=ident[:])
            zrow = constp.tile([P, 1220], f32)
            nc.gpsimd.memset(zrow[:], 0.0)

            wt3 = wpool.tile([P, 8, F], f16, name="wt3")
            bt = miscp.tile([P, F], f16)

            dA = dpool.tile([P, DW], f16, tag="dA")
            dB = dpool.tile([P, DW], f16, tag="dB")
            for t in (dA, dB):
                nc.gpsimd.memset(t[:, 0:HALO], 0.0)
                nc.gpsimd.memset(t[:, HALO + F:DW], 0.0)

            # ---------------- preprocessing ----------------
            with tc.tile_pool(name="pre", bufs=2) as prep, \
                 tc.tile_pool(name="psumpre", bufs=4, space="PSUM") as psumpre:
                # wrap-column masks synthesized on device: F mod W = 912, so
                # zero columns cycle period-4 in p. Ones via Pool memsets;
                # zeros via tiny strided SWDGE DMAs from the zero tile.
                maskLt = prep.tile([P, F], f16, tag="mL", bufs=1)
                maskRt = prep.tile([P, F], f16, tag="mR", bufs=1)
                zrow16 = zrow[0:1, 0:48].bitcast(f16)
                nc.gpsimd.memset(maskLt[:], 1.0)
                nc.gpsimd.memset(maskRt[:], 1.0)
                for r in range(4):
                    for mt, j0 in ((maskLt, (-r * F) % W),
                                   (maskRt, (W - 1 - r * F) % W)):
                        cols = [j0 + k * W for k in range(3) if j0 + k * W < F]
                        st = mt[r:r + 1, 0:1]
                        dst = bass_rust.AP(
                            tensor=st.tensor, offset=st.offset + cols[0],
                            ap=[[4 * st.ap[0][0], (P - r + 3) // 4],
                                [W, len(cols)]])
                        nparts = (P - r + 3) // 4
                        nc.gpsimd.dma_start(
                            out=dst,
                            in_=zrow16[0:1, 0:nparts * len(cols)])
                sparse_st = prep.tile([P, F], f32, tag="sp32", bufs=1)
                blur_st = prep.tile([P, F], f32, tag="bl32", bufs=1)
                sign16 = prep.tile([P, F], f16, tag="m", bufs=1)
                m16inv = prep.tile([P, F], f16, tag="minv", bufs=1)

                # wsum accumulated on PE from masked gate planes
                psw = [psumpre.tile([P, CP], f32, name=f"psw{q}", bufs=1,
                                    tag=f"psw{q}") for q in range(8)]
                # blur/sparse first: d0 and the sign -> (1-m) chain complete
                # early, far off the guidance-load critical path
                nc.sync.dma_start(
                    out=blur_st[:],
                    in_=blur[:].rearrange("(p f) -> p f", p=P))
                nc.scalar.dma_start(
                    out=sparse_st[:],
                    in_=sparse[:].rearrange("(p f) -> p f", p=P))
                # d0 = blur (fp16 body via ACT; halos via the SWDGE queue so
                # they cannot head-of-line block the guidance loads)
                nc.scalar.copy(out=dA[:, HALO:HALO + F], in_=blur_st[:])
                nc.gpsimd.dma_start(out=dA[1:128, 0:HALO],
                                    in_=dA[0:127, F:F + HALO])
                nc.gpsimd.dma_start(out=dA[0:127, HALO + F:DW],
                                    in_=dA[1:128, HALO:2 * HALO])
                nc.scalar.sign(sign16[:], sparse_st[:])
                nc.vector.tensor_scalar(m16inv[:], sign16[:], -1.0,
                                        1.0, MULT, ADD)
                nc.vector.tensor_tensor(bt[:], sign16[:], blur_st[:], MULT)

                # load order: the 6 wrap-masked dirs first (grpB then +-1),
                # the two mask-free dirs (dx=0, +-1216) last, column-chunk
                # interleaved so each wsum bank completes as its chunk lands
                order = [4, 5, 1, 2, 6, 7]  # wt3 positions, stripe-loaded
                npool_masks = 0
                for i, pos in enumerate(order):
                    s = WSHIFTS[pos]
                    gst = prep.tile([P, F], f32, tag="gst", bufs=3)
                    _emit_shifted_plane_load(nc, gst, g, CHAN_FOR_SHIFT[s], s,
                                             zrow)
                    # |g| -> fp16 gate plane in wt3
                    nc.scalar.activation(wt3[:, pos, :], gst[:],
                                         mybir.ActivationFunctionType.Abs)
                    dx = 1 if s in (-1215, 1, 1217) else \
                        (-1 if s in (-1217, -1, 1215) else 0)
                    if dx != 0:
                        mt = maskLt if dx == -1 else maskRt
                        # two of the six wrap masks run on the idle Pool
                        if npool_masks < 2:
                            npool_masks += 1
                            nc.gpsimd.tensor_tensor(
                                wt3[:, pos, :], wt3[:, pos, :], mt[:], MULT)
                        else:
                            nc.vector.tensor_tensor(
                                wt3[:, pos, :], wt3[:, pos, :], mt[:], MULT)
                    for q in range(8):
                        qs = q * CP
                        nc.tensor.matmul(psw[q][:], identt[:],
                                         wt3[:, pos, qs:qs + CP],
                                         start=(i == 0), stop=False)

                # last two planes: shifts -1216 (pos 0) and +1216 (pos 3),
                # both mask-free, loaded per column chunk, interleaved
                gstA = prep.tile([P, F], f32, tag="gst", bufs=3, name="gstA")
                gstB = prep.tile([P, F], f32, tag="gst", bufs=3, name="gstB")
                engs = (nc.sync, nc.scalar)
                nq = 0
                chanA = CHAN_FOR_SHIFT[-1216]
                chanB = CHAN_FOR_SHIFT[1216]
                for cc in range(NCHUNK):
                    lo, hi = cc * CD, (cc + 1) * CD
                    # --- plane A: shift -1216 into wt3 pos 0 ---
                    uA = g[chanA, 0:1]
                    srcA = bass_rust.AP(
                        tensor=uA.tensor, offset=uA.offset + (F - 1216) + lo,
                        ap=[[F, 127], [1, CD]])
                    engs[nq % 2].dma_start(out=gstA[1:128, lo:hi], in_=srcA)
                    nq += 1
                    # partition 0: dest j reads source j-1216 (zero if j<1216)
                    z1 = min(hi, 1216)
                    if lo < z1:
                        nc.vector.memset(gstA[0:1, lo:z1], 0.0)
                    if hi > 1216:
                        a0 = max(lo, 1216)
                        engs[nq % 2].dma_start(
                            out=gstA[0:1, a0:hi],
                            in_=g[chanA, a0 - 1216:hi - 1216].rearrange(
                                "(p f) -> p f", p=1))
                        nq += 1
                    nc.scalar.activation(
                        wt3[:, 0, lo:hi], gstA[:, lo:hi],
                        mybir.ActivationFunctionType.Abs)
                    for q in range(2 * cc, 2 * cc + 2):
                        qs = q * CP
                        nc.tensor.matmul(psw[q][:], identt[:],
                                         wt3[:, 0, qs:qs + CP],
                                         start=False, stop=False)
                    # --- plane B: shift +1216 into wt3 pos 3 ---
                    sB = 1216
                    uB = g[chanB, 0:1]
                    srcB = bass_rust.AP(
                        tensor=uB.tensor, offset=uB.offset + sB + lo,
                        ap=[[F, 127], [1, CD]])
                    engs[nq % 2].dma_start(out=gstB[0:127, lo:hi], in_=srcB)
                    nq += 1
                    lim = F - sB  # 2128: partition 127 valid source bound
                    if lo < lim:
                        v = min(hi, lim)
                        u2 = g[chanB, 0:1]
                        src2 = bass_rust.AP(
                            tensor=u2.tensor,
                            offset=u2.offset + sB + 127 * F + lo,
                            ap=[[F, 1], [1, v - lo]])
                        engs[nq % 2].dma_start(
                            out=gstB[127:128, lo:v], in_=src2)
                        nq += 1
                        if v < hi:
                            engs[nq % 2].dma_start(
                                out=gstB[127:128, v:hi],
                                in_=zrow[0:1, 0:hi - v])
                            nq += 1
                    else:
                        engs[nq % 2].dma_start(
                            out=gstB[127:128, lo:hi], in_=zrow[0:1, 0:CD])
                        nq += 1
                    nc.scalar.activation(
                        wt3[:, 3, lo:hi], gstB[:, lo:hi],
                        mybir.ActivationFunctionType.Abs)
                    for q in range(2 * cc, 2 * cc + 2):
                        qs = q * CP
                        nc.tensor.matmul(psw[q][:], identt[:],
                                         wt3[:, 3, qs:qs + CP],
                                         start=False, stop=True)

                # winv = (1-mask)/wsum, fp16. 1/wsum on ACT (Reciprocal
                # activation) so DVE only does the (1-m) product and the
                # weight scaling; all per-bank so iteration 1 can start
                # under the input-DMA shadow.
                winv = prep.tile([P, F], f16, tag="winv", bufs=1)
                for q in range(8):
                    sl = slice(q * CP, (q + 1) * CP)
                    with nc.allow_low_precision(
                            reason="fp16 weights absorb 1/wsum"):
                        nc.vector.reciprocal(winv[:, sl], psw[q][:])
                    nc.vector.tensor_tensor(winv[:, sl], winv[:, sl],
                                            m16inv[:, sl], MULT)
                    # W_k *= winv, all 8 planes: split so DVE (0.52ns/elem)
                    # and Pool (1.98ns/elem) finish together
                    wd = 333
                    nc.vector.tensor_tensor(
                        _ap4(wt3, q * CP, F, 8, 1, 1, wd),
                        _ap4(wt3, q * CP, F, 8, 1, 1, wd),
                        _ap4(winv, q * CP, 0, 8, 1, 1, wd), MULT)
                    nc.gpsimd.tensor_tensor(
                        _ap4(wt3, q * CP + wd, F, 8, 1, 1, CP - wd),
                        _ap4(wt3, q * CP + wd, F, 8, 1, 1, CP - wd),
                        _ap4(winv, q * CP + wd, 0, 8, 1, 1, CP - wd), MULT)

            # ---------------- stencil iterations ----------------
            with tc.tile_pool(name="prod", bufs=3) as prodp, \
                 tc.tile_pool(name="psum", bufs=6, space="PSUM") as psump, \
                 tc.tile_pool(name="post", bufs=1) as postp:

                src, dst = dA, dB
                ostage = postp.tile([P, F], f32)
                even_pieces = [(0, CD), (CD, CD), (2 * CD, CD), (3 * CD, CD)]
                odd_pieces = [(3 * CD, CD), (2 * CD, CD), (CD, CD), (0, CD)]
                for it in range(prop_time):
                    last = it == prop_time - 1
                    pieces = even_pieces if it % 2 == 0 else odd_pieces
                    back_done = front_done = False
                    covered = []
                    for (cs, cw) in pieces:
                        pr = prodp.tile([P, 8, cw], f16,
                                        tag=f"pr{cw}", name=f"pr{cw}")
                        # Pool: grpB right part (far shifts only -> never
                        # waits on the freshest casts)
                        nc.gpsimd.tensor_tensor(
                            _ap4(pr, 4 * cw + WE, cw, 2, 2 * cw, 2, cw - WE),
                            _ap4(wt3, 4 * F + cs + WE, F, 2, 2 * F, 2,
                                 cw - WE),
                            _ap4(src, HALO - 1217 + cs + WE, 2, 2, 2432, 2,
                                 cw - WE), MULT)
                        # DVE: grpB left part
                        nc.vector.tensor_tensor(
                            _ap4(pr, 4 * cw, cw, 2, 2 * cw, 2, WE),
                            _ap4(wt3, 4 * F + cs, F, 2, 2 * F, 2, WE),
                            _ap4(src, HALO - 1217 + cs, 2, 2, 2432, 2, WE),
                            MULT)
                        # DVE: grpA (includes +-1 -> waits on same-chunk cast)
                        nc.vector.tensor_tensor(
                            _ap4(pr, 0, cw, 2, 2 * cw, 2, cw),
                            _ap4(wt3, cs, F, 2, 2 * F, 2, cw),
                            _ap4(src, HALO - 1216 + cs, 1215, 2, 1217, 2, cw),
                            MULT)
                        for h in range(cw // CP):
                            hs = h * CP
                            ps = psump.tile([P, CP], f32)
                            # seed the bank with b on ACT; matmuls accumulate
                            nc.scalar.copy(out=ps[:],
                                           in_=bt[:, cs + hs:cs + hs + CP])
                            for j, k in enumerate((4, 5, 6, 7, 0, 1, 2, 3)):
                                nc.tensor.matmul(ps[:], identt[:],
                                                 pr[:, k, hs:hs + CP],
                                                 start=False, stop=(j == 7))
                            if last:
                                nc.scalar.copy(
                                    out=ostage[:, cs + hs:cs + hs + CP],
                                    in_=ps[:])
                                nc.sync.dma_start(
                                    out=out[:, cs + hs:cs + hs + CP],
                                    in_=ostage[:, cs + hs:cs + hs + CP])
                            else:
                                nc.scalar.copy(
                                    out=dst[:, HALO + cs + hs:HALO + cs + hs + CP],
                                    in_=ps[:])
                        if last:
                            continue
                        covered.append((cs, cs + cw))
                        def _covers(lo, hi):
                            pts = sorted(covered)
                            cur = lo
                            for a, b_ in pts:
                                if a <= cur < b_:
                                    cur = b_
                                if cur >= hi:
                                    return True
                            return cur >= hi
                        # halo refreshes as soon as their source cols land
                        if not back_done and _covers(0, HALO):
                            nc.sync.dma_start(
                                out=dst[0:127, HALO + F:DW],
                                in_=dst[1:128, HALO:2 * HALO])
                            back_done = True
                        if not front_done and _covers(F - HALO, F):
                            nc.scalar.dma_start(
                                out=dst[1:128, 0:HALO],
                                in_=dst[0:127, F:F + HALO])
                            front_done = True
                    src, dst = dst, src

    nc.finalize()
    _split_sync_waits(nc)
    return nc


def _consts():
    return np.eye(P, dtype=np.float16)


def kernel(guidance, blur_depth, sparse_depth):
    if "nc" not in _CACHE:
        _CACHE["nc"] = _build()
    nc = _CACHE["nc"]
    guidance = np.asarray(guidance, dtype=np.float32)
    blur_depth = np.asarray(blur_depth, dtype=np.float32)
    sparse_depth = np.asarray(sparse_depth, dtype=np.float32)
    idm = _consts()
    in_maps = []
    for c in range(B):
        in_maps.append({
            "g": np.ascontiguousarray(guidance[c].reshape(8, HW)),
            "blur": np.ascontiguousarray(blur_depth[c].reshape(HW)),
            "sparse": np.ascontiguousarray(sparse_depth[c].reshape(HW)),
            "ident": idm,
        })
    # every iterate is a convex combination of blur_depth values, so the
    # output must stay inside blur's range; violations mean the device
    # glitched (transient NRT wedge) -> retry
    lo = float(blur_depth.min()) - 1e-2
    hi = float(blur_depth.max()) + 1e-2

    import time
    outp = None
    for attempt in range(4):
        try:
            res = run_bass_kernel_spmd(nc, in_maps, list(range(B)))
            outp = np.stack(
                [res.results[c]["out"].reshape(1, H, W) for c in range(B)])
            if np.isfinite(outp).all() and outp.min() >= lo and outp.max() <= hi:
                return outp
            print(f"kernel: attempt {attempt} produced out-of-range values; "
                  f"retrying", flush=True)
        except Exception as e:
            if attempt == 3:
                raise
            print(f"kernel: attempt {attempt} failed ({type(e).__name__}); "
                  f"retrying", flush=True)
        time.sleep(20 * (attempt + 1))
    return outp


# revision 8
# speedup vs baseline: 1.1733x; 1.1733x over previous
"""AffinityPropagate Trainium2 kernel, v4.

24 iterations of an 8-neighbor gated stencil:
    d <- (1-mask) * sum_k(gsh_k * shift_k(d)) / wsum + mask * blur

Strategy (8 NeuronCores, pure data parallel: one batch image per core):
  * Image [352, 1216] flattened row-major into SBUF [128 part x 3344].
  * Zero-padded 2D shifts == flat 1D shifted reads: per-direction gate
    weights are exactly 0 wherever a neighbor is out of bounds, so the
    row-wrap values the flat shift drags in are annihilated.
  * Per-direction weights W_k = |g_k shifted| * (1-mask)/wsum (fp16) are
    precomputed once into one [128, 8, F] tile, plane order
    [TL: -1217,-1216,-1215 | TR: +1215,+1216,+1217 | C: -1, +1].
  * Products per 836-col chunk as direction-SIDED instructions so the
    cross-iteration wavefront pipelines (a one-sided instruction never
    waits on the freshest casts of its own side):
      - DVE: TL triple [P,3,605] + TR triple [P,3,605] (3-dim APs, fp16
        2x mode) + the C pair [P,2,836];
      - Pool: the right 231 columns of each triple (its Multiply runs at
        0.42 roofline, so it carries ~28% of the triples).
    Engine queues are emitted in a hand-scheduled ladder per iteration
    parity so no engine ever waits at the iteration boundary.
  * PE accumulates the 8 product planes onto a b-seeded PSUM bank via
    identity matmuls (start=False); ACT seeds each bank with b and casts
    PSUM -> fp16 d tiles; +-1218-element halos refresh via 2 SBUF-SBUF
    DMAs per iteration on the idle DMA rings.
"""

import numpy as np

import bass_rust
from concourse import bass, mybir
from concourse.bass_utils import run_bass_kernel_spmd
from concourse.tile import TileContext

B, H, W = 8, 352, 1216
HW = H * W            # 428032
P = 128
F = HW // P           # 3344
HALO = 1218           # > max |shift| (1217), even
DW = HALO + F + HALO  # d tile width
PROP_TIME = 24
NCHUNK = 4            # product chunks per iteration
CD = F // NCHUNK      # 836
CP = 418              # one PSUM bank
TD = 605              # DVE's share of each triple's columns (Pool: rest)

# wt3 plane order (storage): TL = planes 0-2, TR = 3-5, C = 6-7
WSHIFTS = [-1217, -1216, -1215, 1215, 1216, 1217, -1, 1]
# guidance channel k for each reference offset (dy*W+dx), reference order
REF_SHIFTS = [1217, 1216, 1215, 1, -1, -1215, -1216, -1217]
CHAN_FOR_SHIFT = {s: k for k, s in enumerate(REF_SHIFTS)}

f32 = mybir.dt.float32
f16 = mybir.dt.float16
MULT = mybir.AluOpType.mult
ADD = mybir.AluOpType.add

_CACHE = {}


def _split_sync_waits(nc, max_waits=1):
    """The walrus in this container accepts at most one sync-wait command
    per instruction; hoist extras onto preceding same-engine no-ops."""
    for f in nc.m.functions:
        for bb in f.blocks:
            out = []
            for inst in bb.instructions:
                si = inst.sync_info
                if si is not None and si.on_wait and len(si.on_wait) > max_waits:
                    waits = list(si.on_wait)
                    carry, keep = waits[:-max_waits], waits[-max_waits:]
                    for j, w in enumerate(carry):
                        out.append(mybir.InstNoOp(
                            name=f"{inst.name}-ws{j}", engine=inst.engine,
                            sync_info=mybir.SyncInfo(on_wait=[w], on_update=[]),
                            bass_nofuse=True))
                    inst.sync_info = mybir.SyncInfo(
                        on_wait=keep, on_update=list(si.on_update))
                out.append(inst)
            bb.instructions[:] = out


def _ap3(tile, base, d1, n1, w):
    """3-dim AP over `tile` at free offset `base`: [P, n1(d1), w(1)]."""
    s = tile[:]
    return bass_rust.AP(tensor=s.tensor, offset=s.offset + base,
                        ap=[[s.ap[0][0], P], [d1, n1], [1, w]])


def _ap4(tile, base, d1, n1, d2, n2, w):
    s = tile[:]
    return bass_rust.AP(tensor=s.tensor, offset=s.offset + base,
                        ap=[[s.ap[0][0], P], [d1, n1], [d2, n2], [1, w]])


def _emit_shifted_plane_load(nc, gst, g, k, s, zrow):
    """gst[p, j] <- g[k, p*F + j + s], with rows whose 2D source row is out
    of bounds forced to zero (wrap columns are handled via mask multiplies)."""
    engs = (nc.sync, nc.scalar)
    eng = engs[k % 2]
    if s >= 0:
        for i, (p0, p1) in enumerate(((0, 32), (32, 64), (64, 96), (96, 127))):
            engs[(k + i) % 2].dma_start(
                out=gst[p0:p1, :],
                in_=g[k, s + p0 * F:s + p1 * F].rearrange(
                    "(p f) -> p f", p=p1 - p0))
        if s > 0:
            eng.dma_start(
                out=gst[127:128, 0:F - s],
                in_=g[k, s + 127 * F:HW].rearrange("(p f) -> p f", p=1))
        else:
            eng.dma_start(
                out=gst[127:128, :],
                in_=g[k, 127 * F:HW].rearrange("(p f) -> p f", p=1))
    else:
        a = -s
        eng.dma_start(
            out=gst[0:1, a:F],
            in_=g[k, 0:F - a].rearrange("(p f) -> p f", p=1))
        for i, (p0, p1) in enumerate(((1, 32), (32, 64), (64, 96), (96, 128))):
            engs[(k + i) % 2].dma_start(
                out=gst[p0:p1, :],
                in_=g[k, p0 * F - a:p1 * F - a].rearrange(
                    "(p f) -> p f", p=p1 - p0))
    # top/bottom image rows (dy out of bounds) + DMA-uncovered slivers.
    if s in (-1217, -1216, -1215):          # dy = -1
        nc.vector.memset(gst[0:1, 0:max(1216, -s)], 0.0)
    elif s == -1:
        nc.vector.memset(gst[0:1, 0:1], 0.0)
    elif s in (1215, 1216, 1217):           # dy = +1
        start = min(F - 1216, F - s)
        eng.dma_start(out=gst[127:128, start:F], in_=zrow[0:1, 0:F - start])
    elif s == 1:
        eng.dma_start(out=gst[127:128, F - 1:F], in_=zrow[0:1, 0:1])


def _build(prop_time=PROP_TIME):
    nc = bass.Bass()
    g = nc.dram_tensor("g", [8, HW], f32, kind="ExternalInput")
    blur = nc.dram_tensor("blur", [HW], f32, kind="ExternalInput")
    sparse = nc.dram_tensor("sparse", [HW], f32, kind="ExternalInput")
    ident = nc.dram_tensor("ident", [P, P], f16, kind="ExternalInput")
    out = nc.dram_tensor("out", [P, F], f32, kind="ExternalOutput")

    with TileContext(nc) as tc:
        with tc.tile_pool(name="const", bufs=1) as constp, \
             tc.tile_pool(name="wpool", bufs=1) as wpool, \
             tc.tile_pool(name="dpool", bufs=1) as dpool, \
             tc.tile_pool(name="misc", bufs=1) as miscp:

            identt = constp.tile([P, P], f16)
            nc.sync.dma_start(out=identt[:], in_=ident[:])
            zrow = constp.tile([P, 1220], f32)
            nc.gpsimd.memset(zrow[:], 0.0)

            wt3 = wpool.tile([P, 8, F], f16, name="wt3")
            bt = miscp.tile([P, F], f16)

            dA = dpool.tile([P, DW], f16, tag="dA")
            dB = dpool.tile([P, DW], f16, tag="dB")
            for t in (dA, dB):
                nc.gpsimd.memset(t[:, 0:HALO], 0.0)
                nc.gpsimd.memset(t[:, HALO + F:DW], 0.0)

            # ---------------- preprocessing ----------------
            with tc.tile_pool(name="pre", bufs=2) as prep, \
                 tc.tile_pool(name="psumpre", bufs=4, space="PSUM") as psumpre:
                # wrap-column masks synthesized on device: F mod W = 912, so
                # zero columns cycle period-4 in p. Ones via Pool memsets;
                # zeros via tiny strided SWDGE DMAs from the zero tile.
                maskLt = prep.tile([P, F], f16, tag="mL", bufs=1)
                maskRt = prep.tile([P, F], f16, tag="mR", bufs=1)
                zrow16 = zrow[0:1, 0:48].bitcast(f16)
                nc.gpsimd.memset(maskLt[:], 1.0)
                nc.gpsimd.memset(maskRt[:], 1.0)
                for r in range(4):
                    for mt, j0 in ((maskLt, (-r * F) % W),
                                   (maskRt, (W - 1 - r * F) % W)):
                        cols = [j0 + k * W for k in range(3) if j0 + k * W < F]
                        st = mt[r:r + 1, 0:1]
                        dst = bass_rust.AP(
                            tensor=st.tensor, offset=st.offset + cols[0],
                            ap=[[4 * st.ap[0][0], (P - r + 3) // 4],
                                [W, len(cols)]])
                        nparts = (P - r + 3) // 4
                        nc.gpsimd.dma_start(
                            out=dst,
                            in_=zrow16[0:1, 0:nparts * len(cols)])
                sparse_st = prep.tile([P, F], f32, tag="sp32", bufs=1)
                blur_st = prep.tile([P, F], f32, tag="bl32", bufs=1)
                sign16 = prep.tile([P, F], f16, tag="m", bufs=1)
                m16inv = prep.tile([P, F], f16, tag="minv", bufs=1)

                # wsum accumulated on PE from masked gate planes
                psw = [psumpre.tile([P, CP], f32, name=f"psw{q}", bufs=1,
                                    tag=f"psw{q}") for q in range(8)]
                # blur/sparse first: d0 and the sign -> (1-m) chain complete
                # early, far off the guidance-load critical path
                nc.sync.dma_start(
                    out=blur_st[:],
                    in_=blur[:].rearrange("(p f) -> p f", p=P))
                nc.scalar.dma_start(
                    out=sparse_st[:],
                    in_=sparse[:].rearrange("(p f) -> p f", p=P))
                # d0 = blur (fp16 body via ACT; halos via the SWDGE queue so
                # they cannot head-of-line block the guidance loads)
                nc.scalar.copy(out=dA[:, HALO:HALO + F], in_=blur_st[:])
                nc.gpsimd.dma_start(out=dA[1:128, 0:HALO],
                                    in_=dA[0:127, F:F + HALO])
                nc.gpsimd.dma_start(out=dA[0:127, HALO + F:DW],
                                    in_=dA[1:128, HALO:2 * HALO])
                nc.scalar.sign(sign16[:], sparse_st[:])
                nc.vector.tensor_scalar(m16inv[:], sign16[:], -1.0,
                                        1.0, MULT, ADD)
                nc.vector.tensor_tensor(bt[:], sign16[:], blur_st[:], MULT)

                # load order: the 6 wrap-masked dirs first, the two no-mask
                # dirs (dx=0: -+1216, planes 1 and 4) last, column-chunk
                # interleaved so each wsum bank completes as its chunk lands
                order = [0, 2, 3, 5, 6, 7]  # wt3 positions, stripe-loaded
                npool_masks = 0
                for i, pos in enumerate(order):
                    s = WSHIFTS[pos]
                    gst = prep.tile([P, F], f32, tag="gst", bufs=3)
                    _emit_shifted_plane_load(nc, gst, g, CHAN_FOR_SHIFT[s], s,
                                             zrow)
                    # |g| -> fp16 gate plane in wt3
                    nc.scalar.activation(wt3[:, pos, :], gst[:],
                                         mybir.ActivationFunctionType.Abs)
                    dx = 1 if s in (-1215, 1, 1217) else \
                        (-1 if s in (-1217, -1, 1215) else 0)
                    if dx != 0:
                        mt = maskLt if dx == -1 else maskRt
                        # two of the six wrap masks run on the idle Pool
                        if npool_masks < 2:
                            npool_masks += 1
                            nc.gpsimd.tensor_tensor(
                                wt3[:, pos, :], wt3[:, pos, :], mt[:], MULT)
                        else:
                            nc.vector.tensor_tensor(
                                wt3[:, pos, :], wt3[:, pos, :], mt[:], MULT)
                    for q in range(8):
                        qs = q * CP
                        nc.tensor.matmul(psw[q][:], identt[:],
                                         wt3[:, pos, qs:qs + CP],
                                         start=(i == 0), stop=False)

                # last two planes: shifts -1216 (pos 1) and +1216 (pos 4),
                # both mask-free, loaded per column chunk, interleaved
                gstA = prep.tile([P, F], f32, tag="gst", bufs=3, name="gstA")
                gstB = prep.tile([P, F], f32, tag="gst", bufs=3, name="gstB")
                engs = (nc.sync, nc.scalar)
                nq = 0
                chanA = CHAN_FOR_SHIFT[-1216]
                chanB = CHAN_FOR_SHIFT[1216]
                for cc in range(NCHUNK):
                    lo, hi = cc * CD, (cc + 1) * CD
                    # --- plane A: shift -1216 into wt3 pos 1 ---
                    uA = g[chanA, 0:1]
                    srcA = bass_rust.AP(
                        tensor=uA.tensor, offset=uA.offset + (F - 1216) + lo,
                        ap=[[F, 127], [1, CD]])
                    engs[nq % 2].dma_start(out=gstA[1:128, lo:hi], in_=srcA)
                    nq += 1
                    # partition 0: dest j reads source j-1216 (zero if j<1216)
                    z1 = min(hi, 1216)
                    if lo < z1:
                        nc.vector.memset(gstA[0:1, lo:z1], 0.0)
                    if hi > 1216:
                        a0 = max(lo, 1216)
                        engs[nq % 2].dma_start(
                            out=gstA[0:1, a0:hi],
                            in_=g[chanA, a0 - 1216:hi - 1216].rearrange(
                                "(p f) -> p f", p=1))
                        nq += 1
                    nc.scalar.activation(
                        wt3[:, 1, lo:hi], gstA[:, lo:hi],
                        mybir.ActivationFunctionType.Abs)
                    for q in range(2 * cc, 2 * cc + 2):
                        qs = q * CP
                        nc.tensor.matmul(psw[q][:], identt[:],
                                         wt3[:, 1, qs:qs + CP],
                                         start=False, stop=False)
                    # --- plane B: shift +1216 into wt3 pos 4 ---
                    sB = 1216
                    uB = g[chanB, 0:1]
                    srcB = bass_rust.AP(
                        tensor=uB.tensor, offset=uB.offset + sB + lo,
                        ap=[[F, 127], [1, CD]])
                    engs[nq % 2].dma_start(out=gstB[0:127, lo:hi], in_=srcB)
                    nq += 1
                    lim = F - sB  # 2128: partition 127 valid source bound
                    if lo < lim:
                        v = min(hi, lim)
                        u2 = g[chanB, 0:1]
                        src2 = bass_rust.AP(
                            tensor=u2.tensor,
                            offset=u2.offset + sB + 127 * F + lo,
                            ap=[[F, 1], [1, v - lo]])
                        engs[nq % 2].dma_start(
                            out=gstB[127:128, lo:v], in_=src2)
                        nq += 1
                        if v < hi:
                            engs[nq % 2].dma_start(
                                out=gstB[127:128, v:hi],
                                in_=zrow[0:1, 0:hi - v])
                            nq += 1
                    else:
                        engs[nq % 2].dma_start(
                            out=gstB[127:128, lo:hi], in_=zrow[0:1, 0:CD])
                        nq += 1
                    nc.scalar.activation(
                        wt3[:, 4, lo:hi], gstB[:, lo:hi],
                        mybir.ActivationFunctionType.Abs)
                    for q in range(2 * cc, 2 * cc + 2):
                        qs = q * CP
                        nc.tensor.matmul(psw[q][:], identt[:],
                                         wt3[:, 4, qs:qs + CP],
                                         start=False, stop=True)

                # winv = (1-mask)/wsum (fp16); then W_k *= winv per bank,
                # all 8 planes in one DVE + one Pool instruction per bank
                winv = prep.tile([P, F], f16, tag="winv", bufs=1)
                for q in range(8):
                    sl = slice(q * CP, (q + 1) * CP)
                    with nc.allow_low_precision(
                            reason="fp16 weights absorb 1/wsum"):
                        nc.vector.reciprocal(winv[:, sl], psw[q][:])
                    nc.vector.tensor_tensor(winv[:, sl], winv[:, sl],
                                            m16inv[:, sl], MULT)
                    wd = 333  # DVE 0.52ns/elem vs Pool 1.98: finish together
                    nc.vector.tensor_tensor(
                        _ap3(wt3, q * CP, F, 8, wd),
                        _ap3(wt3, q * CP, F, 8, wd),
                        _ap3(winv, q * CP, 0, 8, wd), MULT)
                    nc.gpsimd.tensor_tensor(
                        _ap3(wt3, q * CP + wd, F, 8, CP - wd),
                        _ap3(wt3, q * CP + wd, F, 8, CP - wd),
                        _ap3(winv, q * CP + wd, 0, 8, CP - wd), MULT)

            # ---------------- stencil iterations ----------------
            with tc.tile_pool(name="prod", bufs=6) as prodp, \
                 tc.tile_pool(name="psum", bufs=6, space="PSUM") as psump, \
                 tc.tile_pool(name="post", bufs=1) as postp:

                src, dst = dA, dB
                ostage = postp.tile([P, F], f32)

                # Hand-scheduled per-engine ladders (chunk indices), chosen
                # so every instruction's cross-iteration dependency is
                # satisfied by the time the engine reaches it. Mirrored
                # between parities.  TL/TR = triples, C = the +-1 pair.
                dve_desc = [("TL", 3), ("TR", 3), ("TL", 2), ("C", 2),
                            ("C", 3), ("TR", 2), ("C", 1), ("TR", 1),
                            ("TL", 1), ("TL", 0), ("TR", 0), ("C", 0)]
                dve_asc = [("TR", 0), ("TL", 0), ("TR", 1), ("C", 1),
                           ("C", 0), ("TL", 1), ("C", 2), ("TL", 2),
                           ("TR", 2), ("TR", 3), ("TL", 3), ("C", 3)]
                pool_desc = [("TL", 3), ("TR", 3), ("TL", 2), ("TR", 2),
                             ("TR", 1), ("TL", 1), ("TL", 0), ("TR", 0)]
                pool_asc = [("TR", 0), ("TL", 0), ("TR", 1), ("TL", 1),
                            ("TL", 2), ("TR", 2), ("TR", 3), ("TL", 3)]
                mm_desc = [3, 2, 1, 0]
                mm_asc = [0, 1, 2, 3]

                def emit_product(kind, pr, c, eng):
                    """Emit one product instruction for chunk c into pr[c]."""
                    cs = c * CD
                    if kind == "C":  # +-1 pair, DVE only, full width
                        eng.tensor_tensor(
                            _ap3(pr, 6 * CD, CD, 2, CD),
                            _ap3(wt3, 6 * F + cs, F, 2, CD),
                            _ap3(src, HALO - 1 + cs, 2, 2, CD), MULT)
                        return
                    pl = 0 if kind == "TL" else 3
                    sh = -1217 if kind == "TL" else 1215
                    if eng is nc.vector:
                        w0, w1 = 0, TD
                    else:
                        w0, w1 = TD, CD
                    eng.tensor_tensor(
                        _ap3(pr, pl * CD + w0, CD, 3, w1 - w0),
                        _ap3(wt3, pl * F + cs + w0, F, 3, w1 - w0),
                        _ap3(src, HALO + sh + cs + w0, 1, 3, w1 - w0), MULT)

                for it in range(prop_time):
                    last = it == prop_time - 1
                    desc = it % 2 == 1
                    dve_sched = dve_desc if desc else dve_asc
                    pool_sched = pool_desc if desc else pool_asc
                    mm_sched = mm_desc if desc else mm_asc

                    prs = {c: prodp.tile([P, 8, CD], f16, tag="pr",
                                         name=f"pr{it}c{c}")
                           for c in mm_sched}
                    # interleave: pool ladder first (its SEQ dispatches
                    # early), then DVE ladder; tile deps do the rest
                    for kind, c in pool_sched:
                        emit_product(kind, prs[c], c, nc.gpsimd)
                    for kind, c in dve_sched:
                        emit_product(kind, prs[c], c, nc.vector)

                    back_done = front_done = False
                    done_chunks = []
                    for c in mm_sched:
                        cs = c * CD
                        pr = prs[c]
                        # within-chunk bank order mirrors the parity so the
                        # +-1 pair's fresh-cast dependency lands one bank
                        # early on both parities
                        for h in ((1, 0) if desc else (0, 1)):
                            hs = h * CP
                            ps = psump.tile([P, CP], f32)
                            # seed the bank with b on ACT; matmuls accumulate
                            nc.scalar.copy(out=ps[:],
                                           in_=bt[:, cs + hs:cs + hs + CP])
                            for k in range(8):
                                nc.tensor.matmul(ps[:], identt[:],
                                                 pr[:, k, hs:hs + CP],
                                                 start=False, stop=(k == 7))
                            if last:
                                nc.scalar.copy(
                                    out=ostage[:, cs + hs:cs + hs + CP],
                                    in_=ps[:])
                                nc.sync.dma_start(
                                    out=out[:, cs + hs:cs + hs + CP],
                                    in_=ostage[:, cs + hs:cs + hs + CP])
                            else:
                                nc.scalar.copy(
                                    out=dst[:, HALO + cs + hs:HALO + cs + hs + CP],
                                    in_=ps[:])
                        if last:
                            continue
                        done_chunks.append(c)
                        # halo refreshes as soon as their source cols land
                        if not back_done and {0, 1} <= set(done_chunks):
                            nc.sync.dma_start(
                                out=dst[0:127, HALO + F:DW],
                                in_=dst[1:128, HALO:2 * HALO])
                            back_done = True
                        if not front_done and {2, 3} <= set(done_chunks):
                            nc.scalar.dma_start(
                                out=dst[1:128, 0:HALO],
                                in_=dst[0:127, F:F + HALO])
                            front_done = True
                    src, dst = dst, src

    nc.finalize()
    _split_sync_waits(nc)
    return nc


def _consts():
    return np.eye(P, dtype=np.float16)


def kernel(guidance, blur_depth, sparse_depth):
    if "nc" not in _CACHE:
        _CACHE["nc"] = _build()
    nc = _CACHE["nc"]
    guidance = np.asarray(guidance, dtype=np.float32)
    blur_depth = np.asarray(blur_depth, dtype=np.float32)
    sparse_depth = np.asarray(sparse_depth, dtype=np.float32)
    idm = _consts()
    in_maps = []
    for c in range(B):
        in_maps.append({
            "g": np.ascontiguousarray(guidance[c].reshape(8, HW)),
            "blur": np.ascontiguousarray(blur_depth[c].reshape(HW)),
            "sparse": np.ascontiguousarray(sparse_depth[c].reshape(HW)),
            "ident": idm,
        })
    # every iterate is a convex combination of blur_depth values, so the
    # output must stay inside blur's range; violations mean the device
    # glitched (transient NRT wedge) -> retry
    lo = float(blur_depth.min()) - 1e-2
    hi = float(blur_depth.max()) + 1e-2

    import time
    outp = None
    for attempt in range(4):
        try:
            res = run_bass_kernel_spmd(nc, in_maps, list(range(B)))
            outp = np.stack(
                [res.results[c]["out"].reshape(1, H, W) for c in range(B)])
            if np.isfinite(outp).all() and outp.min() >= lo and outp.max() <= hi:
                return outp
            print(f"kernel: attempt {attempt} produced out-of-range values; "
                  f"retrying", flush=True)
        except Exception as e:
            if attempt == 3:
                raise
            print(f"kernel: attempt {attempt} failed ({type(e).__name__}); "
                  f"retrying", flush=True)
        time.sleep(20 * (attempt + 1))
    return outp


# revision 34
# speedup vs baseline: 1.1913x; 1.0153x over previous
"""AffinityPropagate Trainium2 kernel, v2.

24 iterations of an 8-neighbor gated stencil:
    d <- (1-mask) * sum_k(gsh_k * shift_k(d)) / wsum + mask * blur

Strategy (8 NeuronCores, pure data parallel: one batch image per core):
  * Image [352, 1216] flattened row-major into SBUF [128 part x 3344].
  * Zero-padded 2D shifts == flat 1D shifted reads: per-direction gate
    weights are exactly 0 wherever a neighbor is out of bounds, so the
    row-wrap values the flat shift drags in are annihilated.
  * Per-direction weights W_k = |g_k shifted| * (1-mask)/wsum (fp16) are
    precomputed once into one [128, 8, F] tile.
  * Per iteration (4 column-chunks of 836, direction alternating to keep
    the cross-iteration dependency wavefront short): DVE computes 6 product
    planes as 3 two-direction pair instructions (3-dim overlapping APs keep
    fp16 2x mode) plus ~30% of a 7th; GpSimd (Pool) computes the 8th plane
    and ~70% of the 7th; ACT seeds each PSUM bank with b, PE accumulates
    the 8 product planes onto it via identity matmuls (start=False), ACT
    casts PSUM->fp16 into ping-pong d tiles; the +-1218-element halos are
    refreshed with 2 SBUF->SBUF DMAs per iteration.
  * The cost model charges DVE 2x mode for packed fp16 regardless of byte
    alignment, so no aligned d_odd copy is kept. Walrus rejects
    TensorScalarPtr on Pool, so Pool uses plain tensor_tensor.
"""

import numpy as np

import bass_rust
from concourse import bass, mybir
from concourse.bass_utils import run_bass_kernel_spmd
from concourse.tile import TileContext

B, H, W = 8, 352, 1216
HW = H * W            # 428032
P = 128
F = HW // P           # 3344
HALO = 1218           # > max |shift| (1217), even
DW = HALO + F + HALO  # d tile width
PROP_TIME = 24
NCHUNK = 4            # product chunks per iteration
CD = F // NCHUNK      # 836
CP = 418              # one PSUM bank
NSUB = CD // CP       # 2

# storage order of weight planes in wt3 (first 7 on DVE, last on Pool);
# DVE pairs: (0,1) d-delta 1, (2,3) d-delta 2, (4,5) d-delta 1; single: 6
WSHIFTS = [-1216, -1215, -1, 1, 1215, 1216, 1217, -1217]
# guidance channel k for each reference offset (dy*W+dx), reference order
REF_SHIFTS = [1217, 1216, 1215, 1, -1, -1215, -1216, -1217]
CHAN_FOR_SHIFT = {s: k for k, s in enumerate(REF_SHIFTS)}
DVE_PAIRS = [(0, 1), (2, 3), (4, 5)]
DVE_SINGLE = 6
POOL_DIR = 7

f32 = mybir.dt.float32
f16 = mybir.dt.float16
MULT = mybir.AluOpType.mult
ADD = mybir.AluOpType.add

_CACHE = {}


def _split_sync_waits(nc, max_waits=1):
    """The walrus in this container accepts at most one sync-wait command
    per instruction; hoist extras onto preceding same-engine no-ops."""
    for f in nc.m.functions:
        for bb in f.blocks:
            out = []
            for inst in bb.instructions:
                si = inst.sync_info
                if si is not None and si.on_wait and len(si.on_wait) > max_waits:
                    waits = list(si.on_wait)
                    carry, keep = waits[:-max_waits], waits[-max_waits:]
                    for j, w in enumerate(carry):
                        out.append(mybir.InstNoOp(
                            name=f"{inst.name}-ws{j}", engine=inst.engine,
                            sync_info=mybir.SyncInfo(on_wait=[w], on_update=[]),
                            bass_nofuse=True))
                    inst.sync_info = mybir.SyncInfo(
                        on_wait=keep, on_update=list(si.on_update))
                out.append(inst)
            bb.instructions[:] = out


def _pair_read_ap(dtile, start, delta, width):
    """AP reading dtile[:, start : start+width] and
    dtile[:, start+delta : start+delta+width] as a [P, 2, width] view."""
    s = dtile[:, start:start + width]
    pstride = s.ap[0][0]
    return bass_rust.AP(tensor=s.tensor, offset=s.offset,
                        ap=[[pstride, P], [delta, 2], [1, width]])


def _bcast_k_ap(tile2d, start, width, nk):
    """AP reading tile2d[:, start:start+width] broadcast over a middle
    k-dim of size nk -> [P, nk, width] view."""
    s = tile2d[:, start:start + width]
    pstride = s.ap[0][0]
    return bass_rust.AP(tensor=s.tensor, offset=s.offset,
                        ap=[[pstride, P], [0, nk], [1, width]])


def _emit_shifted_plane_load(nc, gst, g, k, s, zrow):
    """gst[p, j] <- g[k, p*F + j + s], with rows whose 2D source row is out
    of bounds forced to zero (wrap columns are handled via mask multiplies)."""
    engs = (nc.sync, nc.scalar)
    eng = engs[k % 2]
    if s >= 0:
        for i, (p0, p1) in enumerate(((0, 32), (32, 64), (64, 96), (96, 127))):
            engs[(k + i) % 2].dma_start(
                out=gst[p0:p1, :],
                in_=g[k, s + p0 * F:s + p1 * F].rearrange(
                    "(p f) -> p f", p=p1 - p0))
        if s > 0:
            eng.dma_start(
                out=gst[127:128, 0:F - s],
                in_=g[k, s + 127 * F:HW].rearrange("(p f) -> p f", p=1))
        else:
            eng.dma_start(
                out=gst[127:128, :],
                in_=g[k, 127 * F:HW].rearrange("(p f) -> p f", p=1))
    else:
        a = -s
        eng.dma_start(
            out=gst[0:1, a:F],
            in_=g[k, 0:F - a].rearrange("(p f) -> p f", p=1))
        for i, (p0, p1) in enumerate(((1, 32), (32, 64), (64, 96), (96, 128))):
            engs[(k + i) % 2].dma_start(
                out=gst[p0:p1, :],
                in_=g[k, p0 * F - a:p1 * F - a].rearrange(
                    "(p f) -> p f", p=p1 - p0))
    # top/bottom image rows (dy out of bounds) + DMA-uncovered slivers.
    if s in (-1217, -1216, -1215):          # dy = -1
        nc.vector.memset(gst[0:1, 0:max(1216, -s)], 0.0)
    elif s == -1:
        nc.vector.memset(gst[0:1, 0:1], 0.0)
    elif s in (1215, 1216, 1217):           # dy = +1
        start = min(F - 1216, F - s)
        eng.dma_start(out=gst[127:128, start:F], in_=zrow[0:1, 0:F - start])
    elif s == 1:
        eng.dma_start(out=gst[127:128, F - 1:F], in_=zrow[0:1, 0:1])


def _build():
    nc = bass.Bass()
    g = nc.dram_tensor("g", [8, HW], f32, kind="ExternalInput")
    blur = nc.dram_tensor("blur", [HW], f32, kind="ExternalInput")
    sparse = nc.dram_tensor("sparse", [HW], f32, kind="ExternalInput")
    ident = nc.dram_tensor("ident", [P, P], f16, kind="ExternalInput")
    out = nc.dram_tensor("out", [P, F], f32, kind="ExternalOutput")

    with TileContext(nc) as tc:
        with tc.tile_pool(name="const", bufs=1) as constp, \
             tc.tile_pool(name="wpool", bufs=1) as wpool, \
             tc.tile_pool(name="dpool", bufs=1) as dpool, \
             tc.tile_pool(name="misc", bufs=1) as miscp:

            identt = constp.tile([P, P], f16)
            nc.sync.dma_start(out=identt[:], in_=ident[:])
            zrow = constp.tile([P, 1220], f32)
            nc.gpsimd.memset(zrow[:], 0.0)

            wt3 = wpool.tile([P, 8, F], f16, name="wt3")
            bt = miscp.tile([P, F], f16)

            dA = dpool.tile([P, DW], f16, tag="dA")
            dB = dpool.tile([P, DW], f16, tag="dB")
            for t in (dA, dB):
                nc.gpsimd.memset(t[:, 0:HALO], 0.0)
                nc.gpsimd.memset(t[:, HALO + F:DW], 0.0)

            # ---------------- preprocessing ----------------
            with tc.tile_pool(name="pre", bufs=2) as prep, \
                 tc.tile_pool(name="psumpre", bufs=4, space="PSUM") as psumpre:
                # wrap-column masks synthesized on device (saves 4.8us of
                # input DMA): F mod W = 912, so zero columns cycle period-4
                # in p. Ones via Pool memsets; zeros via tiny strided SWDGE
                # DMAs from the zero tile (walrus rejects strided Memset APs
                # but strided DMA destinations are routine).
                maskLt = prep.tile([P, F], f16, tag="mL", bufs=1)
                maskRt = prep.tile([P, F], f16, tag="mR", bufs=1)
                zrow16 = zrow[0:1, 0:48].bitcast(f16)
                nc.gpsimd.memset(maskLt[:], 1.0)
                nc.gpsimd.memset(maskRt[:], 1.0)
                for r in range(4):
                    for mt, j0 in ((maskLt, (-r * F) % W),
                                   (maskRt, (W - 1 - r * F) % W)):
                        cols = [j0 + k * W for k in range(3) if j0 + k * W < F]
                        st = mt[r:r + 1, 0:1]
                        dst = bass_rust.AP(
                            tensor=st.tensor, offset=st.offset + cols[0],
                            ap=[[4 * st.ap[0][0], (P - r + 3) // 4],
                                [W, len(cols)]])
                        nparts = (P - r + 3) // 4
                        nc.gpsimd.dma_start(
                            out=dst,
                            in_=zrow16[0:1, 0:nparts * len(cols)])
                sparse_st = prep.tile([P, F], f32, tag="sp32", bufs=1)
                blur_st = prep.tile([P, F], f32, tag="bl32", bufs=1)
                sign16 = prep.tile([P, F], f16, tag="m", bufs=1)
                m16inv = prep.tile([P, F], f16, tag="minv", bufs=1)

                # wsum accumulated on PE from masked gate planes
                psw = [psumpre.tile([P, CP], f32, name=f"psw{q}", bufs=1,
                                    tag=f"psw{q}") for q in range(8)]
                # blur/sparse first: d0 and the sign -> (1-m) chain complete
                # early, far off the guidance-load critical path
                nc.sync.dma_start(
                    out=blur_st[:],
                    in_=blur[:].rearrange("(p f) -> p f", p=P))
                nc.scalar.dma_start(
                    out=sparse_st[:],
                    in_=sparse[:].rearrange("(p f) -> p f", p=P))
                # d0 = blur (fp16 body via ACT; halos via the SWDGE queue so
                # they cannot head-of-line block the guidance loads)
                nc.scalar.copy(out=dA[:, HALO:HALO + F], in_=blur_st[:])
                nc.gpsimd.dma_start(out=dA[1:128, 0:HALO],
                                    in_=dA[0:127, F:F + HALO])
                nc.gpsimd.dma_start(out=dA[0:127, HALO + F:DW],
                                    in_=dA[1:128, HALO:2 * HALO])
                nc.scalar.sign(sign16[:], sparse_st[:])
                nc.vector.tensor_scalar(m16inv[:], sign16[:], -1.0,
                                        1.0, MULT, ADD)
                nc.vector.tensor_tensor(bt[:], sign16[:], blur_st[:], MULT)

                # load order: Pool-masked dirs first (longest post-chain),
                # the two no-mask dirs (dx=0) last, column-chunk interleaved
                # so each wsum bank's chain completes as its chunk lands
                order = [7, 1, 2, 3, 4, 6]  # wt3 positions, stripe-loaded
                npool_masks = 0
                for i, pos in enumerate(order):
                    s = WSHIFTS[pos]
                    gst = prep.tile([P, F], f32, tag="gst", bufs=3)
                    _emit_shifted_plane_load(nc, gst, g, CHAN_FOR_SHIFT[s], s,
                                             zrow)
                    # |g| -> fp16 gate plane in wt3
                    nc.scalar.activation(wt3[:, pos, :], gst[:],
                                         mybir.ActivationFunctionType.Abs)
                    dx = 1 if s in (-1215, 1, 1217) else \
                        (-1 if s in (-1217, -1, 1215) else 0)
                    if dx != 0:
                        mt = maskLt if dx == -1 else maskRt
                        # two of the six wrap masks run on the idle Pool
                        if npool_masks < 2:
                            npool_masks += 1
                            nc.gpsimd.tensor_tensor(
                                wt3[:, pos, :], wt3[:, pos, :], mt[:], MULT)
                        else:
                            nc.vector.tensor_tensor(
                                wt3[:, pos, :], wt3[:, pos, :], mt[:], MULT)
                    for q in range(8):
                        qs = q * CP
                        nc.tensor.matmul(psw[q][:], identt[:],
                                         wt3[:, pos, qs:qs + CP],
                                         start=(i == 0), stop=False)

                # last two planes: shifts -1216 (pos 0) and +1216 (pos 5),
                # both mask-free, loaded per column chunk, interleaved
                gstA = prep.tile([P, F], f32, tag="gst", bufs=3, name="gstA")
                gstB = prep.tile([P, F], f32, tag="gst", bufs=3, name="gstB")
                engs = (nc.sync, nc.scalar)
                nq = 0
                chanA = CHAN_FOR_SHIFT[-1216]
                chanB = CHAN_FOR_SHIFT[1216]
                for cc in range(NCHUNK):
                    lo, hi = cc * CD, (cc + 1) * CD
                    # --- plane A: shift -1216 into wt3 pos 0 ---
                    uA = g[chanA, 0:1]
                    srcA = bass_rust.AP(
                        tensor=uA.tensor, offset=uA.offset + (F - 1216) + lo,
                        ap=[[F, 127], [1, CD]])
                    engs[nq % 2].dma_start(out=gstA[1:128, lo:hi], in_=srcA)
                    nq += 1
                    # partition 0: dest j reads source j-1216 (zero if j<1216)
                    z1 = min(hi, 1216)
                    if lo < z1:
                        nc.vector.memset(gstA[0:1, lo:z1], 0.0)
                    if hi > 1216:
                        a0 = max(lo, 1216)
                        engs[nq % 2].dma_start(
                            out=gstA[0:1, a0:hi],
                            in_=g[chanA, a0 - 1216:hi - 1216].rearrange(
                                "(p f) -> p f", p=1))
                        nq += 1
                    nc.scalar.activation(
                        wt3[:, 0, lo:hi], gstA[:, lo:hi],
                        mybir.ActivationFunctionType.Abs)
                    for q in range(2 * cc, 2 * cc + 2):
                        qs = q * CP
                        nc.tensor.matmul(psw[q][:], identt[:],
                                         wt3[:, 0, qs:qs + CP],
                                         start=False, stop=False)
                    # --- plane B: shift +1216 into wt3 pos 5 ---
                    sB = 1216
                    uB = g[chanB, 0:1]
                    srcB = bass_rust.AP(
                        tensor=uB.tensor, offset=uB.offset + sB + lo,
                        ap=[[F, 127], [1, CD]])
                    engs[nq % 2].dma_start(out=gstB[0:127, lo:hi], in_=srcB)
                    nq += 1
                    lim = F - sB  # 2128: partition 127 valid source bound
                    if lo < lim:
                        v = min(hi, lim)
                        u2 = g[chanB, 0:1]
                        src2 = bass_rust.AP(
                            tensor=u2.tensor,
                            offset=u2.offset + sB + 127 * F + lo,
                            ap=[[F, 1], [1, v - lo]])
                        engs[nq % 2].dma_start(
                            out=gstB[127:128, lo:v], in_=src2)
                        nq += 1
                        if v < hi:
                            engs[nq % 2].dma_start(
                                out=gstB[127:128, v:hi],
                                in_=zrow[0:1, 0:hi - v])
                            nq += 1
                    else:
                        engs[nq % 2].dma_start(
                            out=gstB[127:128, lo:hi], in_=zrow[0:1, 0:CD])
                        nq += 1
                    nc.scalar.activation(
                        wt3[:, 5, lo:hi], gstB[:, lo:hi],
                        mybir.ActivationFunctionType.Abs)
                    for q in range(2 * cc, 2 * cc + 2):
                        qs = q * CP
                        nc.tensor.matmul(psw[q][:], identt[:],
                                         wt3[:, 5, qs:qs + CP],
                                         start=False, stop=True)

                # winv' = (1-mask)/wsum, fp16; then W_k *= winv' in place
                winv = prep.tile([P, F], f16, tag="winv", bufs=1)
                for c in range(NCHUNK):
                    sl = slice(c * CD, (c + 1) * CD)
                    with nc.allow_low_precision(
                            reason="fp16 weights absorb 1/wsum"):
                        q0 = c * CD // CP
                        nc.vector.reciprocal(winv[:, q0 * CP:q0 * CP + CP],
                                             psw[q0][:])
                        nc.vector.reciprocal(
                            winv[:, (q0 + 1) * CP:(q0 + 2) * CP],
                            psw[q0 + 1][:])
                    nc.vector.tensor_tensor(winv[:, sl], winv[:, sl],
                                            m16inv[:, sl], MULT)
                    # W_k = gate_k * (1-mask)/wsum  (2 planes per instr;
                    # the Pool-owned plane 7 scales on Pool)
                    for p in range(3):
                        nc.vector.tensor_tensor(
                            wt3[:, 2 * p:2 * p + 2, sl],
                            wt3[:, 2 * p:2 * p + 2, sl],
                            _bcast_k_ap(winv, c * CD, CD, 2), MULT)
                    nc.gpsimd.tensor_tensor(
                        wt3[:, 6, sl], wt3[:, 6, sl], winv[:, sl], MULT)
                    nc.gpsimd.tensor_tensor(
                        wt3[:, 7, sl], wt3[:, 7, sl], winv[:, sl], MULT)

            # ---------------- 24 stencil iterations ----------------
            with tc.tile_pool(name="prod", bufs=3) as prodp, \
                 tc.tile_pool(name="psum", bufs=6, space="PSUM") as psump, \
                 tc.tile_pool(name="post", bufs=1) as postp:

                src, dst = dA, dB
                ostage = postp.tile([P, F], f32)
                sp = WSHIFTS[POOL_DIR]
                ss = WSHIFTS[DVE_SINGLE]
                # per-iteration piece list (col_start, width): the final
                # chunk splits into two PSUM-bank halves so the next
                # iteration's first products wait on a shorter ACT tail
                even_pieces = [(0, CD), (CD, CD), (2 * CD, CD), (3 * CD, CD)]
                odd_pieces = [(3 * CD, CD), (2 * CD, CD), (CD, CD), (0, CD)]
                for it in range(PROP_TIME):
                    last = it == PROP_TIME - 1
                    pieces = even_pieces if it % 2 == 0 else odd_pieces
                    back_done = front_done = False
                    covered = []
                    for (cs, cw) in pieces:
                        pr = prodp.tile([P, 8, cw], f16,
                                        tag=f"pr{cw}", name=f"pr{cw}")
                        # Pool owns dir 7 everywhere and dir 6 on half of
                        # the columns; DVE covers dir 6's other half
                        nc.gpsimd.tensor_tensor(
                            pr[:, POOL_DIR, :], wt3[:, POOL_DIR, cs:cs + cw],
                            src[:, HALO + sp + cs:HALO + sp + cs + cw], MULT)
                        h6 = (cw * 29) // 41  # ~0.707: equalize DVE/Pool
                        nc.gpsimd.tensor_tensor(
                            pr[:, DVE_SINGLE, 0:h6],
                            wt3[:, DVE_SINGLE, cs:cs + h6],
                            src[:, HALO + ss + cs:HALO + ss + cs + h6], MULT)
                        def _frag():
                            nc.vector.tensor_tensor(
                                pr[:, DVE_SINGLE, h6:cw],
                                wt3[:, DVE_SINGLE, cs + h6:cs + cw],
                                src[:, HALO + ss + cs + h6:
                                    HALO + ss + cs + cw], MULT)
                        # +1217 reads the far-forward window: ready first on
                        # even iterations, last on odd ones
                        if it % 2 == 0:
                            _frag()
                        pairs = DVE_PAIRS if it % 2 == 0 else DVE_PAIRS[::-1]
                        for (k1, k2) in pairs:
                            d1, d2 = WSHIFTS[k1], WSHIFTS[k2]
                            nc.vector.tensor_tensor(
                                pr[:, k1:k1 + 2, :],
                                wt3[:, k1:k1 + 2, cs:cs + cw],
                                _pair_read_ap(src, HALO + d1 + cs,
                                              d2 - d1, cw), MULT)
                        if it % 2 == 1:
                            _frag()
                        for h in range(cw // CP):
                            hs = h * CP
                            ps = psump.tile([P, CP], f32)
                            # seed the bank with b on ACT; matmuls accumulate
                            nc.scalar.copy(out=ps[:],
                                           in_=bt[:, cs + hs:cs + hs + CP])
                            for k in range(8):
                                nc.tensor.matmul(ps[:], identt[:],
                                                 pr[:, k, hs:hs + CP],
                                                 start=False, stop=(k == 7))
                            if last:
                                nc.scalar.copy(
                                    out=ostage[:, cs + hs:cs + hs + CP],
                                    in_=ps[:])
                                nc.sync.dma_start(
                                    out=out[:, cs + hs:cs + hs + CP],
                                    in_=ostage[:, cs + hs:cs + hs + CP])
                            else:
                                nc.scalar.copy(
                                    out=dst[:, HALO + cs + hs:HALO + cs + hs + CP],
                                    in_=ps[:])
                        if last:
                            continue
                        covered.append((cs, cs + cw))
                        def _covers(lo, hi):
                            pts = sorted(covered)
                            cur = lo
                            for a, b_ in pts:
                                if a <= cur < b_:
                                    cur = b_
                                if cur >= hi:
                                    return True
                            return cur >= hi
                        # halo refreshes as soon as their source cols land
                        if not back_done and _covers(0, HALO):
                            nc.sync.dma_start(
                                out=dst[0:127, HALO + F:DW],
                                in_=dst[1:128, HALO:2 * HALO])
                            back_done = True
                        if not front_done and _covers(F - HALO, F):
                            nc.scalar.dma_start(
                                out=dst[1:128, 0:HALO],
                                in_=dst[0:127, F:F + HALO])
                            front_done = True
                    src, dst = dst, src

    nc.finalize()
    _split_sync_waits(nc)
    return nc


def _consts():
    return np.eye(P, dtype=np.float16)


def kernel(guidance, blur_depth, sparse_depth):
    if "nc" not in _CACHE:
        _CACHE["nc"] = _build()
    nc = _CACHE["nc"]
    guidance = np.asarray(guidance, dtype=np.float32)
    blur_depth = np.asarray(blur_depth, dtype=np.float32)
    sparse_depth = np.asarray(sparse_depth, dtype=np.float32)
    idm = _consts()
    in_maps = []
    for c in range(B):
        in_maps.append({
            "g": np.ascontiguousarray(guidance[c].reshape(8, HW)),
            "blur": np.ascontiguousarray(blur_depth[c].reshape(HW)),
            "sparse": np.ascontiguousarray(sparse_depth[c].reshape(HW)),
            "ident": idm,
        })
    # every iterate is a convex combination of blur_depth values, so the
    # output must stay inside blur's range; violations mean the device
    # glitched (transient NRT wedge) -> retry
    lo = float(blur_depth.min()) - 1e-2
    hi = float(blur_depth.max()) + 1e-2

    import time
    outp = None
    for attempt in range(4):
        try:
            res = run_bass_kernel_spmd(nc, in_maps, list(range(B)))
            outp = np.stack(
                [res.results[c]["out"].reshape(1, H, W) for c in range(B)])
            if np.isfinite(outp).all() and outp.min() >= lo and outp.max() <= hi:
                return outp
            print(f"kernel: attempt {attempt} produced out-of-range values; "
                  f"retrying", flush=True)
        except Exception as e:
            if attempt == 3:
                raise
            print(f"kernel: attempt {attempt} failed ({type(e).__name__}); "
                  f"retrying", flush=True)
        time.sleep(20 * (attempt + 1))
    return outp

